# revision 36
# baseline (speedup 1.0000x reference)
"""Trainium2 Bass kernel for channel (cross-covariance) self-attention.

Shapes (hardcoded): x (8, 4096, 512) f32, wqkv_w (1536, 512), wqkv_b (1536,),
wp_w (512, 512), wp_b (512,). NUM_HEADS=8, head_dim=64.

Sharding: data-parallel over batch b across the 8 NeuronCores (one batch
element per core). Weights replicated (host pre-transposed + fp16-cast).

Per-core algorithm (fp16 data path, fp32 PSUM accumulation):
  - All inputs are host-side pre-transposed/cast; every device load is a
    plain contiguous DMA (no xbar transpose engine) on one HWDGE ring in
    criticality order (x0/wq ct0 first), so the first qk matmul starts as
    soon as the first ~400KB land.  x is fully SBUF-resident (32KB/part).
  - Column biases are broadcast on-device with rank-1 matmuls
    (ones^T @ bias_row) instead of shipping 512KB of replicated rows.
  - qk[t, f] (f in 0:1024) via stationary xT tiles against moving fp16
    weights, evacuated in [128,1024] PSUM pairs with fused bias add on DVE.
  - scores_h[d,e] = sum q_s^T k_s in fp16, col-packed even/odd streams.
    Big and small matmuls are kept in contiguous runs (each big<->small
    transition costs ~100ns of PE pipeline restart).
  - softmax with no max-subtraction (|scores/8| < ~45, exp stays in fp32
    range) shortens the DVE/ACT chain so it hides under the v matmuls.
  - vT[vc, t] via stationary fp16 weight slices; the ACT-evac (bias, fp16
    cast) writes straight into the permuted V' layout
    V'T[h*64+e, u*512+q] = v[h*512+q, u*64+e], so the old separate y-phase
    (block-diag attention matmuls + assembly) is gone entirely.
  - Instead, the softmax weights are folded into the output projection:
    WP'[h*64+e, f] = sum_d softmax_h[d,e] * wp[f, h*64+d]; both heads of a
    pair are computed as two concurrent col-tiled 64-contraction matmuls.
  - out = V'^T-tiles @ WP' in permuted token order; stores are [128,2,512]
    tiles into a [p, mt, f]-major HBM tensor; the host un-permutes (free).
"""

import numpy as np
from contextlib import ExitStack

import concourse.bass as bass
import concourse.tile as tile
from concourse import bacc, mybir
from concourse.bass_utils import run_bass_kernel_spmd
from concourse.masks import make_identity

dt = mybir.dt

N_TOK = 4096
C = 512
H = 8
D = 64
TB = 512          # tokens per head block
SUB = TB // 128   # 4 token tiles per head block
CT = C // 128     # 4 contraction tiles

_cache = {}


def _emit(ctx: ExitStack, tc, out_d, x_d, wqkT_d, wvT_d, wpT_d, brow_d, vb_d):
    nc = tc.nc
    f32, f16 = dt.float32, dt.float16
    Ident = mybir.ActivationFunctionType.Identity

    const = ctx.enter_context(tc.tile_pool(name="const", bufs=1))
    headp = ctx.enter_context(tc.tile_pool(name="headp", bufs=2))
    smallp = ctx.enter_context(tc.tile_pool(name="smallp", bufs=2))
    outp = ctx.enter_context(tc.tile_pool(name="outp", bufs=3))
    ps_pair = ctx.enter_context(tc.tile_pool(name="ps_pair", bufs=2, space="PSUM"))
    ps_big = ctx.enter_context(tc.tile_pool(name="ps_big", bufs=2, space="PSUM"))
    ps_sm = ctx.enter_context(tc.tile_pool(name="ps_sm", bufs=1, space="PSUM"))

    # ---------------- SBUF residents ----------------
    xT = const.tile([128, H, CT, TB], f16)       # [p, h, ct, t] = x[h*TB+t, ct*128+p]
    wqkT = const.tile([128, CT, 2 * C], f16)     # [p, ct, f] = wqkv[f, ct*128+p]
    wvT = const.tile([128, CT, C], f16)          # [p, ci, vc] = wqkv[2C+vc, ci*128+p]
    wpT = const.tile([128, CT, C], f16)          # [p, j, f] = wp[f, j*128+p]
    brow = const.tile([1, 4 * C], f16)           # [qk bias 1024 | wp bias x2 1024]
    vb = const.tile([128, CT], f16)              # v bias, partitioned
    biases = const.tile([128, 4 * C], f16)       # brow broadcast to 128 parts
    VT = const.tile([128, CT, N_TOK], f16)       # V'T[c' = j*128+p, u*512+q]
    WPP = const.tile([128, CT, C], f16)          # WP'[c' = j*128+p, f]

    qk_bias = biases[:, 0:2 * C]
    wp_bias2 = biases[:, 2 * C:4 * C]
    vbias = vb

    # ---------------- input DMAs ----------------
    # Single HWDGE ring (sync), FIFO at the SDMA level: the critical first
    # wave (bias row + x0 + wqkT) gets the full DMA bandwidth, everything
    # else streams behind it while head 0 computes.  The scalar ring is
    # kept empty so it doesn't steal packet slots during this window.
    nc.scalar.dma_start(brow, brow_d)            # tiny; scalar ring
    nc.scalar.dma_start(vb, vb_d)
    nc.sync.dma_start(xT[:, 0, 0], x_d[:, 0, 0])     # ct0 slice first: the
    nc.sync.dma_start(wqkT[:, 0], wqkT_d[:, 0])      # first qk MMs need only
    nc.sync.dma_start(xT[:, 0, 1:CT], x_d[:, 0, 1:CT])   # x0/wq ct=0
    for ct in range(1, CT):
        nc.sync.dma_start(wqkT[:, ct], wqkT_d[:, ct])
    nc.sync.dma_start(xT[:, 1], x_d[:, 1])
    nc.sync.dma_start(wvT, wvT_d)
    nc.sync.dma_start(wpT, wpT_d)
    nc.sync.dma_start(xT[:, 2:H], x_d[:, 2:H])   # one 3MB DMA for blocks 2-7

    # ---------------- PE warm-up + bias broadcast ----------------
    # Keep the PE busy from the end of the fixed preamble until the first
    # qk matmul's inputs land, so HAM reaches K=8/8 and never re-throttles.
    # The warm matmuls are FULL-SIZE (128-contraction, 512-wide moving):
    # small matmuls don't register enough activity for the HAM to
    # un-throttle, leaving the first ~10 real matmuls at the 1.2GHz clock.
    # The column biases are broadcast to all 128 partitions with rank-1
    # matmuls (ones^T @ bias_row) instead of shipping 512KB over DMA.
    wsrc = const.tile([128, 512], f16)
    nc.vector.memset(wsrc, 0.0)
    ones = const.tile([1, 128], f16)
    nc.vector.memset(ones, 1.0)
    warm = ps_big.tile([128, TB], f32, tag="ps")
    for wi in range(12):
        nc.tensor.matmul(warm, wsrc[:, 0:128], wsrc, start=True, stop=True)
    for part in range(2):                        # qk part, wp part
        bps = ps_pair.tile([128, 2 * C], f32, tag="pspair")
        for s in range(2):
            nc.tensor.matmul(bps[:, s * C:(s + 1) * C], ones,
                             brow[0:1, (2 * part + s) * C:(2 * part + s + 1) * C],
                             start=True, stop=True)
        # evac on ACT so the DVE stays clear for the first qk-bias ADDs
        nc.scalar.activation(biases[:, 2 * part * C:2 * (part + 1) * C],
                             bps, Ident)

    # ---------------- per-head pipeline ----------------
    # WP'_h[e, f] = sum_d wn[d, e] * wp[f, h*64+d].  Both heads of a pair
    # are computed as two concurrent col-tiled 64-contraction matmuls
    # (disjoint quadrants + banks), deferred into the NEXT head's scores
    # block so they sit inside a small-matmul run (no extra PE restarts).
    wn_pairs = []

    def emit_wpp_pair(jj):
        wn_pair = wn_pairs[jj]
        wpp_ps = ps_pair.tile([128, 2 * C], f32, tag="pspair")
        nc.tensor.matmul(wpp_ps[0:64, 0:C], wn_pair[0:64, :],
                         wpT[0:64, jj, :], start=True, stop=True,
                         tile_position=(0, 0))
        nc.tensor.matmul(wpp_ps[64:128, C:2 * C], wn_pair[64:128, :],
                         wpT[64:128, jj, :], start=True, stop=True,
                         tile_position=(64, 64))
        nc.vector.tensor_copy(WPP[0:64, jj, :], wpp_ps[0:64, 0:C])
        nc.vector.tensor_copy(WPP[64:128, jj, :], wpp_ps[64:128, C:2 * C])

    for h in range(H):
        pb = (h % 2) * 64
        j = h // 2

        # qk[t, f]: stationary xT tiles, moving fp16 weights; psum pairs.
        # Scores groups are interleaved one step behind the qk groups (group
        # i's scores run right after evac i lands) so the softmax chain can
        # start ~2us earlier and complete under the v matmuls.
        qk = headp.tile([128, SUB, 2 * C], f16, tag="qk")
        sc = ps_sm.tile([128, 2, C], f32, tag="pss")
        sc_e = sc[0:64, 0, 0:64]
        sc_o = sc[64:128, 1, 0:64]
        npair = SUB * (H // 2)

        def scores_group(i):
            for sp in range(H // 2):
                s0, s1 = 2 * sp, 2 * sp + 1
                k = i * (H // 2) + sp
                nc.tensor.matmul(
                    sc_e,
                    qk[:, i, s0 * D:(s0 + 1) * D],
                    qk[:, i, C + s0 * D: C + (s0 + 1) * D],
                    start=(k == 0), stop=(k == npair - 1),
                    tile_position=(0, 0))
                nc.tensor.matmul(
                    sc_o,
                    qk[:, i, s1 * D:(s1 + 1) * D],
                    qk[:, i, C + s1 * D: C + (s1 + 1) * D],
                    start=(k == 0), stop=(k == npair - 1),
                    tile_position=(0, 64))

        # vT group ct, evacuated directly into the permuted V' layout:
        # pv rows r<64 -> u=2ct (e=r); rows r>=64 -> u=2ct+1 (e=r-64).
        def v_group(ct):
            pv = ps_big.tile([128, TB], f32, tag="ps")
            for ci in range(CT):
                nc.tensor.matmul(
                    pv,
                    wvT[:, ci, ct * 128:(ct + 1) * 128],
                    xT[:, h, ci, :],
                    start=(ci == 0), stop=(ci == CT - 1))
            for half in range(2):
                u = 2 * ct + half
                nc.scalar.activation(
                    VT[pb:pb + 64, j, u * TB:(u + 1) * TB],
                    pv[half * 64:half * 64 + 64, :],
                    Ident, bias=vbias[half * 64:half * 64 + 64, ct:ct + 1])

        # PE order: all qk groups (big MMs, contiguous), all scores groups
        # (small MMs, contiguous), all v groups, wpp.  Keeping the big and
        # small matmuls in contiguous runs avoids the ~100ns PE pipeline
        # restart that every big<->small transition costs.
        for i in range(SUB):
            pq = ps_pair.tile([128, 2 * C], f32, tag="pspair")
            for ct in range(CT):
                for g in range(2):
                    nc.tensor.matmul(
                        pq[:, g * C:(g + 1) * C],
                        xT[:, h, ct, i * 128:(i + 1) * 128],
                        wqkT[:, ct, g * C:(g + 1) * C],
                        start=(ct == 0), stop=(ct == CT - 1))
            nc.vector.tensor_add(qk[:, i, :], pq, qk_bias)
        for i in range(SUB):
            scores_group(i)
        if h % 2 == 0 and h >= 2:
            emit_wpp_pair((h - 1) // 2)
        sco = smallp.tile([64, 64], f32, tag="sco")
        nc.vector.tensor_copy(sco, sc_o)
        scf = smallp.tile([64, 64], f32, tag="scf")
        nc.vector.tensor_add(scf, sc_e, sco)

        # softmax over e (free axis); scale 1/sqrt(64) folded into exp.
        # No max-subtraction: |scores/8| < ~45 so exp stays in fp32 range.
        # Emitted before v1-3 so the chain completes on DVE/ACT while the
        # PE runs those matmuls — the WP' matmul then starts stall-free.
        wexp = smallp.tile([64, 64], f32, tag="wexp")
        nc.scalar.activation(wexp, scf, mybir.ActivationFunctionType.Exp,
                             scale=0.125)
        rsum = smallp.tile([64, 1], f32, tag="rsum")
        nc.vector.reduce_sum(rsum, wexp, axis=mybir.AxisListType.X)
        rrec = smallp.tile([64, 1], f32, tag="rrec")
        nc.vector.reciprocal(rrec, rsum)
        if h % 2 == 0:
            wn_pair = smallp.tile([128, 64], f16, tag="wn")
            wn_pairs.append(wn_pair)
        else:
            wn_pair = wn_pairs[-1]
        nc.vector.tensor_scalar_mul(wn_pair[pb:pb + 64, :], wexp, rrec)

        for ct in range(CT):
            v_group(ct)

    # final head pair's WP' (no next head to host it)
    emit_wpp_pair(H // 2 - 1)

    # ---------------- output projection (permuted order) ----------------
    # out_d is [128, 32, C]: out_d[p, mt, f] = out[mt*128+p, f] in permuted
    # token order (host un-permutes).  8 stores of 4 mt-tiles each keep the
    # end-of-kernel semaphore teardown short.
    for mp in range(N_TOK // 256):          # pairs of mt tiles
        pp = ps_pair.tile([128, 2 * C], f32, tag="pspair")
        for half in range(2):
            mt = 2 * mp + half
            for j in range(CT):
                nc.tensor.matmul(
                    pp[:, half * C:(half + 1) * C],
                    VT[:, j, mt * 128:(mt + 1) * 128],
                    WPP[:, j, :],
                    start=(j == 0), stop=(j == CT - 1))
        ob = outp.tile([128, 2 * C], f16, tag="ob")
        if mp == N_TOK // 256 - 1:
            # split the final evac+store so the last bytes ship ~0.6us earlier
            for half in range(2):
                nc.vector.tensor_add(ob[:, half * C:(half + 1) * C],
                                     pp[:, half * C:(half + 1) * C],
                                     wp_bias2[:, half * C:(half + 1) * C])
                eng = nc.sync if half == 0 else nc.scalar
                eng.dma_start(out_d[:, 2 * mp + half, :],
                              ob[:, half * C:(half + 1) * C])
        else:
            nc.vector.tensor_add(ob, pp, wp_bias2)
            eng = nc.sync if mp % 2 == 0 else nc.scalar
            eng.dma_start(out_d[:, 2 * mp:2 * (mp + 1), :], ob)


def _build():
    nc = bacc.Bacc("TRN2", target_bir_lowering=False, debug=False,
                   num_devices=8)
    x_d = nc.dram_tensor("xT", [128, H, CT, TB], dt.float16,
                         kind="ExternalInput").ap()
    wqkT_d = nc.dram_tensor("wqkT", [128, CT, 2 * C], dt.float16,
                            kind="ExternalInput").ap()
    wvT_d = nc.dram_tensor("wvT", [128, CT, C], dt.float16,
                           kind="ExternalInput").ap()
    wpT_d = nc.dram_tensor("wpT", [128, CT, C], dt.float16,
                           kind="ExternalInput").ap()
    brow_d = nc.dram_tensor("brow", [1, 4 * C], dt.float16,
                            kind="ExternalInput").ap()
    vb_d = nc.dram_tensor("vb", [128, CT], dt.float16,
                          kind="ExternalInput").ap()
    out_d = nc.dram_tensor("out", [128, N_TOK // 128, C], dt.float16,
                           kind="ExternalOutput").ap()

    with tile.TileContext(nc) as tc:
        with ExitStack() as ctx:
            _emit(ctx, tc, out_d, x_d, wqkT_d, wvT_d, wpT_d, brow_d, vb_d)
    nc.compile()
    return nc


def _get_nc():
    if "nc" not in _cache:
        _cache["nc"] = _build()
    return _cache["nc"]


def _prep_weights(wqkv_w, wqkv_b, wp_w, wp_b):
    wqkv_w = np.asarray(wqkv_w, np.float32)
    wqkv_b = np.asarray(wqkv_b, np.float32)
    wp_w = np.asarray(wp_w, np.float32)
    wp_b = np.asarray(wp_b, np.float32)
    f16 = np.float16
    # wqkT[p, ct, f] = wqkv[f, ct*128+p]
    wqkT = np.ascontiguousarray(
        wqkv_w[:2 * C].T.reshape(CT, 128, 2 * C).transpose(1, 0, 2)
    ).astype(f16)
    # wvT[p, ci, vc] = wqkv[2C+vc, ci*128+p]
    wvT = np.ascontiguousarray(
        wqkv_w[2 * C:].T.reshape(CT, 128, C).transpose(1, 0, 2)
    ).astype(f16)
    # wpT[p, j, f] = wp[f, j*128+p]
    wpT = np.ascontiguousarray(
        wp_w.T.reshape(CT, 128, C).transpose(1, 0, 2)
    ).astype(f16)
    brow = np.concatenate([wqkv_b[:2 * C], np.tile(wp_b, 2)])[None, :]
    vb = wqkv_b[2 * C:].reshape(CT, 128).T
    return {"wqkT": wqkT, "wvT": wvT, "wpT": wpT,
            "brow": np.ascontiguousarray(brow).astype(f16),
            "vb": np.ascontiguousarray(vb).astype(f16)}


def kernel(x, wqkv_w, wqkv_b, wp_w, wp_b, _trace=False, **_trace_kwargs):
    nc = _get_nc()
    x = np.asarray(x, dtype=np.float32)
    w = _prep_weights(wqkv_w, wqkv_b, wp_w, wp_b)
    in_maps = []
    for i in range(8):
        # xT[p, h, ct, t] = x[i, h*TB+t, ct*128+p]
        xi = x[i].reshape(H, TB, CT, 128).transpose(3, 0, 2, 1)
        in_maps.append(dict(
            w, xT=np.ascontiguousarray(xi).astype(np.float16)))
    res = run_bass_kernel_spmd(nc, in_maps, list(range(8)),
                               trace=_trace, **_trace_kwargs)
    outs = []
    for r in res.results:
        o = r["out"].astype(np.float32)          # [p, mt, f], rows u*512+q
        o = o.transpose(1, 0, 2).reshape(N_TOK, C)   # -> [u*512+q, f]
        outs.append(o.reshape(H, TB, C).transpose(1, 0, 2).reshape(N_TOK, C))
    out = np.stack(outs, axis=0)
    if _trace:
        return out, res
    return out


# revision 38
# speedup vs baseline: 1.0232x; 1.0232x over previous
"""Trainium2 Bass kernel for channel (cross-covariance) self-attention.

Shapes (hardcoded): x (8, 4096, 512) f32, wqkv_w (1536, 512), wqkv_b (1536,),
wp_w (512, 512), wp_b (512,). NUM_HEADS=8, head_dim=64.

Sharding: data-parallel over batch b across the 8 NeuronCores (one batch
element per core). Weights replicated (host pre-transposed + fp16-cast).

Per-core algorithm (fp16 data path, fp32 PSUM accumulation):
  - All inputs are host-side pre-transposed/cast; every device load is a
    plain contiguous DMA (no xbar transpose engine) on one HWDGE ring in
    criticality order (x0/wq ct0 first), so the first qk matmul starts as
    soon as the first ~400KB land.  x is fully SBUF-resident (32KB/part).
  - Column biases are broadcast on-device with rank-1 matmuls
    (ones^T @ bias_row) instead of shipping 512KB of replicated rows.
  - qk[t, f] (f in 0:1024) via stationary xT tiles against moving fp16
    weights, evacuated in [128,1024] PSUM pairs with fused bias add on DVE.
  - scores_h[d,e] = sum q_s^T k_s in fp16, col-packed even/odd streams.
    Big and small matmuls are kept in contiguous runs (each big<->small
    transition costs ~100ns of PE pipeline restart).
  - softmax with no max-subtraction (|scores/8| < ~45, exp stays in fp32
    range) shortens the DVE/ACT chain so it hides under the v matmuls.
  - vT[vc, t] via stationary fp16 weight slices; the ACT-evac (bias, fp16
    cast) writes straight into the permuted V' layout
    V'T[h*64+e, u*512+q] = v[h*512+q, u*64+e], so the old separate y-phase
    (block-diag attention matmuls + assembly) is gone entirely.
  - Instead, the softmax weights are folded into the output projection:
    WP'[h*64+e, f] = sum_d softmax_h[d,e] * wp[f, h*64+d]; both heads of a
    pair are computed as two concurrent col-tiled 64-contraction matmuls.
  - out = V'^T-tiles @ WP' in permuted token order; stores are [128,2,512]
    tiles into a [p, mt, f]-major HBM tensor; the host un-permutes (free).
"""

import numpy as np
from contextlib import ExitStack

import concourse.bass as bass
import concourse.tile as tile
from concourse import bacc, mybir
from concourse.bass_utils import run_bass_kernel_spmd
from concourse.masks import make_identity

dt = mybir.dt

N_TOK = 4096
C = 512
H = 8
D = 64
TB = 512          # tokens per head block
SUB = TB // 128   # 4 token tiles per head block
CT = C // 128     # 4 contraction tiles

_cache = {}


def _emit(ctx: ExitStack, tc, out_d, x_d, wqkT_d, wvT_d, wpT_d, brow_d, vb_d):
    nc = tc.nc
    f32, f16 = dt.float32, dt.float16
    Ident = mybir.ActivationFunctionType.Identity

    const = ctx.enter_context(tc.tile_pool(name="const", bufs=1))
    headp = ctx.enter_context(tc.tile_pool(name="headp", bufs=2))
    smallp = ctx.enter_context(tc.tile_pool(name="smallp", bufs=2))
    outp = ctx.enter_context(tc.tile_pool(name="outp", bufs=3))
    ps_pair = ctx.enter_context(tc.tile_pool(name="ps_pair", bufs=2, space="PSUM"))
    ps_big = ctx.enter_context(tc.tile_pool(name="ps_big", bufs=2, space="PSUM"))
    ps_sm = ctx.enter_context(tc.tile_pool(name="ps_sm", bufs=1, space="PSUM"))

    # ---------------- SBUF residents ----------------
    xT = const.tile([128, H, CT, TB], f16)       # [p, h, ct, t] = x[h*TB+t, ct*128+p]
    wqkT = const.tile([128, CT, 2 * C], f16)     # [p, ct, f] = wqkv[f, ct*128+p]
    wvT = const.tile([128, CT, C], f16)          # [p, ci, vc] = wqkv[2C+vc, ci*128+p]
    wpT = const.tile([128, CT, C], f16)          # [p, j, f] = wp[f, j*128+p]
    brow = const.tile([1, 4 * C], f16)           # [qk bias 1024 | wp bias x2 1024]
    vb = const.tile([128, CT], f16)              # v bias, partitioned
    biases = const.tile([128, 4 * C], f16)       # brow broadcast to 128 parts
    VT = const.tile([128, CT, N_TOK], f16)       # V'T[c' = j*128+p, u*512+q]
    WPP = const.tile([128, CT, C], f16)          # WP'[c' = j*128+p, f]

    qk_bias = biases[:, 0:2 * C]
    wp_bias2 = biases[:, 2 * C:4 * C]
    vbias = vb

    # ---------------- input DMAs ----------------
    # Single HWDGE ring (sync), FIFO at the SDMA level: the critical first
    # wave (bias row + x0 + wqkT) gets the full DMA bandwidth, everything
    # else streams behind it while head 0 computes.  The scalar ring is
    # kept empty so it doesn't steal packet slots during this window.
    nc.scalar.dma_start(brow, brow_d)            # tiny; scalar ring
    nc.scalar.dma_start(vb, vb_d)
    # x0/wq chunks interleaved in the order the first qk group consumes
    # them, so the supply-limited start streams without long PE waits.
    for ct in range(CT):
        nc.sync.dma_start(xT[:, 0, ct], x_d[:, 0, ct])
        nc.sync.dma_start(wqkT[:, ct], wqkT_d[:, ct])
    nc.sync.dma_start(xT[:, 1], x_d[:, 1])
    nc.sync.dma_start(wvT, wvT_d)
    nc.sync.dma_start(wpT, wpT_d)
    nc.sync.dma_start(xT[:, 2:H], x_d[:, 2:H])   # one 3MB DMA for blocks 2-7

    # ---------------- PE warm-up + bias broadcast ----------------
    # Keep the PE busy from the end of the fixed preamble until the first
    # qk matmul's inputs land, so HAM reaches K=8/8 and never re-throttles.
    # The warm matmuls are FULL-SIZE (128-contraction, 512-wide moving):
    # small matmuls don't register enough activity for the HAM to
    # un-throttle, leaving the first ~10 real matmuls at the 1.2GHz clock.
    # The column biases are broadcast to all 128 partitions with rank-1
    # matmuls (ones^T @ bias_row) instead of shipping 512KB over DMA.
    wsrc = const.tile([128, 512], f16)
    nc.vector.memset(wsrc, 0.0)
    ones = const.tile([1, 128], f16)
    nc.vector.memset(ones, 1.0)
    warm = ps_big.tile([128, TB], f32, tag="ps")
    for wi in range(12):
        nc.tensor.matmul(warm, wsrc[:, 0:128], wsrc, start=True, stop=True)
    for part in range(2):                        # qk part, wp part
        bps = ps_pair.tile([128, 2 * C], f32, tag="pspair")
        for s in range(2):
            nc.tensor.matmul(bps[:, s * C:(s + 1) * C], ones,
                             brow[0:1, (2 * part + s) * C:(2 * part + s + 1) * C],
                             start=True, stop=True)
        # evac on ACT so the DVE stays clear for the first qk-bias ADDs
        nc.scalar.activation(biases[:, 2 * part * C:2 * (part + 1) * C],
                             bps, Ident)

    # ---------------- per-head pipeline ----------------
    # WP'_h[e, f] = sum_d wn[d, e] * wp[f, h*64+d].  Both heads of a pair
    # are computed as two concurrent col-tiled 64-contraction matmuls
    # (disjoint quadrants + banks), deferred into the NEXT head's scores
    # block so they sit inside a small-matmul run (no extra PE restarts).
    wn_pairs = []

    def emit_wpp_pair(jj):
        wn_pair = wn_pairs[jj]
        wpp_ps = ps_pair.tile([128, 2 * C], f32, tag="pspair")
        nc.tensor.matmul(wpp_ps[0:64, 0:C], wn_pair[0:64, :],
                         wpT[0:64, jj, :], start=True, stop=True,
                         tile_position=(0, 0))
        nc.tensor.matmul(wpp_ps[64:128, C:2 * C], wn_pair[64:128, :],
                         wpT[64:128, jj, :], start=True, stop=True,
                         tile_position=(64, 64))
        nc.vector.tensor_copy(WPP[0:64, jj, :], wpp_ps[0:64, 0:C])
        nc.vector.tensor_copy(WPP[64:128, jj, :], wpp_ps[64:128, C:2 * C])

    for h in range(H):
        pb = (h % 2) * 64
        j = h // 2

        # qk[t, f]: stationary xT tiles, moving fp16 weights; psum pairs.
        # Scores groups are interleaved one step behind the qk groups (group
        # i's scores run right after evac i lands) so the softmax chain can
        # start ~2us earlier and complete under the v matmuls.
        qk = headp.tile([128, SUB, 2 * C], f16, tag="qk")
        sc = ps_sm.tile([128, 2, C], f32, tag="pss")
        sc_e = sc[0:64, 0, 0:64]
        sc_o = sc[64:128, 1, 0:64]
        npair = SUB * (H // 2)

        def scores_group(i):
            for sp in range(H // 2):
                s0, s1 = 2 * sp, 2 * sp + 1
                k = i * (H // 2) + sp
                nc.tensor.matmul(
                    sc_e,
                    qk[:, i, s0 * D:(s0 + 1) * D],
                    qk[:, i, C + s0 * D: C + (s0 + 1) * D],
                    start=(k == 0), stop=(k == npair - 1),
                    tile_position=(0, 0))
                nc.tensor.matmul(
                    sc_o,
                    qk[:, i, s1 * D:(s1 + 1) * D],
                    qk[:, i, C + s1 * D: C + (s1 + 1) * D],
                    start=(k == 0), stop=(k == npair - 1),
                    tile_position=(0, 64))

        # vT group ct, evacuated directly into the permuted V' layout:
        # pv rows r<64 -> u=2ct (e=r); rows r>=64 -> u=2ct+1 (e=r-64).
        def v_group(ct):
            pv = ps_big.tile([128, TB], f32, tag="ps")
            for ci in range(CT):
                nc.tensor.matmul(
                    pv,
                    wvT[:, ci, ct * 128:(ct + 1) * 128],
                    xT[:, h, ci, :],
                    start=(ci == 0), stop=(ci == CT - 1))
            for half in range(2):
                u = 2 * ct + half
                nc.scalar.activation(
                    VT[pb:pb + 64, j, u * TB:(u + 1) * TB],
                    pv[half * 64:half * 64 + 64, :],
                    Ident, bias=vbias[half * 64:half * 64 + 64, ct:ct + 1])

        # PE order: all qk groups (big MMs, contiguous), all scores groups
        # (small MMs, contiguous), all v groups, wpp.  Keeping the big and
        # small matmuls in contiguous runs avoids the ~100ns PE pipeline
        # restart that every big<->small transition costs.
        for i in range(SUB):
            pq = ps_pair.tile([128, 2 * C], f32, tag="pspair")
            for ct in range(CT):
                for g in range(2):
                    nc.tensor.matmul(
                        pq[:, g * C:(g + 1) * C],
                        xT[:, h, ct, i * 128:(i + 1) * 128],
                        wqkT[:, ct, g * C:(g + 1) * C],
                        start=(ct == 0), stop=(ct == CT - 1))
                if h == 0 and i == 0 and ct < CT - 1:
                    # bridge the DMA-supply gaps of the very first group
                    # with dependency-free warm matmuls so the HAM never
                    # sees enough idle to re-throttle the PE clock
                    for wi in range(2):
                        nc.tensor.matmul(warm, wsrc[:, 0:128], wsrc,
                                         start=True, stop=True)
            nc.vector.tensor_add(qk[:, i, :], pq, qk_bias)
        for i in range(SUB):
            scores_group(i)
        if h % 2 == 0 and h >= 2:
            emit_wpp_pair((h - 1) // 2)
        sco = smallp.tile([64, 64], f32, tag="sco")
        nc.vector.tensor_copy(sco, sc_o)
        scf = smallp.tile([64, 64], f32, tag="scf")
        nc.vector.tensor_add(scf, sc_e, sco)

        # softmax over e (free axis); scale 1/sqrt(64) folded into exp.
        # No max-subtraction: |scores/8| < ~45 so exp stays in fp32 range.
        # Emitted before v1-3 so the chain completes on DVE/ACT while the
        # PE runs those matmuls — the WP' matmul then starts stall-free.
        wexp = smallp.tile([64, 64], f32, tag="wexp")
        nc.scalar.activation(wexp, scf, mybir.ActivationFunctionType.Exp,
                             scale=0.125)
        rsum = smallp.tile([64, 1], f32, tag="rsum")
        nc.vector.reduce_sum(rsum, wexp, axis=mybir.AxisListType.X)
        rrec = smallp.tile([64, 1], f32, tag="rrec")
        nc.vector.reciprocal(rrec, rsum)
        if h % 2 == 0:
            wn_pair = smallp.tile([128, 64], f16, tag="wn")
            wn_pairs.append(wn_pair)
        else:
            wn_pair = wn_pairs[-1]
        nc.vector.tensor_scalar_mul(wn_pair[pb:pb + 64, :], wexp, rrec)

        for ct in range(CT):
            v_group(ct)

    # final head pair's WP' (no next head to host it)
    emit_wpp_pair(H // 2 - 1)

    # ---------------- output projection (permuted order) ----------------
    # out_d is [128, 32, C]: out_d[p, mt, f] = out[mt*128+p, f] in permuted
    # token order (host un-permutes).  8 stores of 4 mt-tiles each keep the
    # end-of-kernel semaphore teardown short.
    for mp in range(N_TOK // 256):          # pairs of mt tiles
        pp = ps_pair.tile([128, 2 * C], f32, tag="pspair")
        for half in range(2):
            mt = 2 * mp + half
            for j in range(CT):
                nc.tensor.matmul(
                    pp[:, half * C:(half + 1) * C],
                    VT[:, j, mt * 128:(mt + 1) * 128],
                    WPP[:, j, :],
                    start=(j == 0), stop=(j == CT - 1))
        ob = outp.tile([128, 2 * C], f16, tag="ob")
        if mp == N_TOK // 256 - 1:
            # split the final evac+store so the last bytes ship ~0.6us earlier
            for half in range(2):
                nc.vector.tensor_add(ob[:, half * C:(half + 1) * C],
                                     pp[:, half * C:(half + 1) * C],
                                     wp_bias2[:, half * C:(half + 1) * C])
                eng = nc.sync if half == 0 else nc.scalar
                eng.dma_start(out_d[:, 2 * mp + half, :],
                              ob[:, half * C:(half + 1) * C])
        else:
            nc.vector.tensor_add(ob, pp, wp_bias2)
            eng = nc.sync if mp % 2 == 0 else nc.scalar
            eng.dma_start(out_d[:, 2 * mp:2 * (mp + 1), :], ob)


def _build():
    nc = bacc.Bacc("TRN2", target_bir_lowering=False, debug=False,
                   num_devices=8)
    x_d = nc.dram_tensor("xT", [128, H, CT, TB], dt.float16,
                         kind="ExternalInput").ap()
    wqkT_d = nc.dram_tensor("wqkT", [128, CT, 2 * C], dt.float16,
                            kind="ExternalInput").ap()
    wvT_d = nc.dram_tensor("wvT", [128, CT, C], dt.float16,
                           kind="ExternalInput").ap()
    wpT_d = nc.dram_tensor("wpT", [128, CT, C], dt.float16,
                           kind="ExternalInput").ap()
    brow_d = nc.dram_tensor("brow", [1, 4 * C], dt.float16,
                            kind="ExternalInput").ap()
    vb_d = nc.dram_tensor("vb", [128, CT], dt.float16,
                          kind="ExternalInput").ap()
    out_d = nc.dram_tensor("out", [128, N_TOK // 128, C], dt.float16,
                           kind="ExternalOutput").ap()

    with tile.TileContext(nc) as tc:
        with ExitStack() as ctx:
            _emit(ctx, tc, out_d, x_d, wqkT_d, wvT_d, wpT_d, brow_d, vb_d)
    nc.compile()
    return nc


def _get_nc():
    if "nc" not in _cache:
        _cache["nc"] = _build()
    return _cache["nc"]


def _prep_weights(wqkv_w, wqkv_b, wp_w, wp_b):
    wqkv_w = np.asarray(wqkv_w, np.float32)
    wqkv_b = np.asarray(wqkv_b, np.float32)
    wp_w = np.asarray(wp_w, np.float32)
    wp_b = np.asarray(wp_b, np.float32)
    f16 = np.float16
    # wqkT[p, ct, f] = wqkv[f, ct*128+p]
    wqkT = np.ascontiguousarray(
        wqkv_w[:2 * C].T.reshape(CT, 128, 2 * C).transpose(1, 0, 2)
    ).astype(f16)
    # wvT[p, ci, vc] = wqkv[2C+vc, ci*128+p]
    wvT = np.ascontiguousarray(
        wqkv_w[2 * C:].T.reshape(CT, 128, C).transpose(1, 0, 2)
    ).astype(f16)
    # wpT[p, j, f] = wp[f, j*128+p]
    wpT = np.ascontiguousarray(
        wp_w.T.reshape(CT, 128, C).transpose(1, 0, 2)
    ).astype(f16)
    brow = np.concatenate([wqkv_b[:2 * C], np.tile(wp_b, 2)])[None, :]
    vb = wqkv_b[2 * C:].reshape(CT, 128).T
    return {"wqkT": wqkT, "wvT": wvT, "wpT": wpT,
            "brow": np.ascontiguousarray(brow).astype(f16),
            "vb": np.ascontiguousarray(vb).astype(f16)}


def kernel(x, wqkv_w, wqkv_b, wp_w, wp_b, _trace=False, **_trace_kwargs):
    nc = _get_nc()
    x = np.asarray(x, dtype=np.float32)
    w = _prep_weights(wqkv_w, wqkv_b, wp_w, wp_b)
    in_maps = []
    for i in range(8):
        # xT[p, h, ct, t] = x[i, h*TB+t, ct*128+p]
        xi = x[i].reshape(H, TB, CT, 128).transpose(3, 0, 2, 1)
        in_maps.append(dict(
            w, xT=np.ascontiguousarray(xi).astype(np.float16)))
    res = run_bass_kernel_spmd(nc, in_maps, list(range(8)),
                               trace=_trace, **_trace_kwargs)
    outs = []
    for r in res.results:
        o = r["out"].astype(np.float32)          # [p, mt, f], rows u*512+q
        o = o.transpose(1, 0, 2).reshape(N_TOK, C)   # -> [u*512+q, f]
        outs.append(o.reshape(H, TB, C).transpose(1, 0, 2).reshape(N_TOK, C))
    out = np.stack(outs, axis=0)
    if _trace:
        return out, res
    return out
